# revision 6
# baseline (speedup 1.0000x reference)
"""ChebyKAN layer on 8 Trainium2 NeuronCores.

y = einsum('dbi,dio->bo', cheby_basis(tanh(x)), cheby_coeffs)

Strategy (per core, data-parallel over batch):
  - each core takes 1024 rows of x (8192/8) and the full coeffs
  - x rows are PE-transposed to [i, b] layout, tanh applied on evacuation
  - Chebyshev basis built on-the-fly in fp32 (DVE), rounded to fp32r
  - contraction as fp32r matmuls (full-rate on TRN2, ~1e-4 rel err):
    stationary = W[d, i-tile, o-tile], moving = T_d[i-tile, b-chunk],
    psum holds y.T chunks [o-tile 128, b-half 512] x 8 o-tiles = 8 banks
  - two b-halves of 512; W streamed from HBM once per half
  - output is y.T per core; host transposes and concatenates
"""

import numpy as np

import concourse.bass as bass
import concourse.tile as tile
from concourse import bacc, mybir
from concourse import bass_utils
from concourse import masks
from concourse.alu_op_type import AluOpType

N_CORES = 8
B = 8192
IC = 1024
OC = 1024
DEG = 8  # polynomial degree; DEG+1 = 9 basis terms
BC = B // N_CORES  # 1024 batch rows per core
P = 128
NI = IC // P  # 8 i-tiles
NO = OC // P  # 8 o-tiles
BH = BC // 2  # 512, b-half
F32 = mybir.dt.float32
F32R = mybir.dt.float32r


def _build(tanh_scale: float, tanh_bias: float):
    nc = bacc.Bacc("TRN2", target_bir_lowering=False, debug=False, num_devices=N_CORES)

    x_d = nc.dram_tensor("x", [BC, IC], F32, kind="ExternalInput").ap()
    w_d = nc.dram_tensor("w", [DEG + 1, IC, OC], F32, kind="ExternalInput").ap()
    yt_d = nc.dram_tensor("yt", [OC, BC], F32, kind="ExternalOutput").ap()

    with tile.TileContext(nc) as tc:
        with (
            tc.tile_pool(name="const", bufs=1) as constp,
            tc.tile_pool(name="xin", bufs=2) as xinp,
            tc.tile_pool(name="xt", bufs=2) as xtp,
            tc.tile_pool(name="state", bufs=3) as statep,
            tc.tile_pool(name="prod", bufs=1) as prodp,
            tc.tile_pool(name="tr", bufs=2) as trp,
            tc.tile_pool(name="wstage", bufs=2) as wstagep,
            tc.tile_pool(name="wr", bufs=2) as wrp,
            tc.tile_pool(name="evac", bufs=2) as evacp,
            tc.tile_pool(name="ps", bufs=8, space=bass.MemorySpace.PSUM) as psp,
        ):
            ident = constp.tile([P, P], F32)
            masks.make_identity(nc, ident[:])
            ones_f = constp.tile([P, BH], F32)
            nc.gpsimd.memset(ones_f[:], 1.0)
            ones_r = constp.tile([P, BH], F32R)
            nc.gpsimd.tensor_copy(ones_r[:], ones_f[:])

            for h in range(2):
                # ---- x transpose + tanh: xt[:, it*BH + b] = tanh(x[b, i]) ----
                # xt free layout: i_tile-major, 512 b-local each
                xt = xtp.tile([P, NI * BH], F32, tag="xt")
                for bt in range(BH // P):  # 4 b-tiles of 128 in this half
                    xnat = xinp.tile([P, IC], F32, tag="xin")
                    nc.sync.dma_start(
                        xnat[:], x_d[h * BH + bt * P : h * BH + (bt + 1) * P, :]
                    )
                    for it in range(NI):
                        ptile = psp.tile([P, 512], F32, tag="ps")
                        nc.tensor.transpose(
                            ptile[:, 0:P], xnat[:, it * P : (it + 1) * P], ident[:]
                        )
                        nc.scalar.activation(
                            xt[:, it * BH + bt * P : it * BH + (bt + 1) * P],
                            ptile[:, 0:P],
                            mybir.ActivationFunctionType.Tanh,
                            bias=tanh_bias,
                            scale=tanh_scale,
                        )

                # ---- accumulation psum tiles: y.T chunk per o-tile ----
                accs = [
                    psp.tile([P, BH], F32, tag="ps", name=f"acc_h{h}_o{ot}")
                    for ot in range(NO)
                ]

                # ---- degree loop ----
                t_m1 = xt  # T_{d-1} (fp32 slab view)
                t_m2 = None  # T_{d-2}
                for d in range(DEG + 1):
                    # fp32r moving operand for this degree
                    if d == 0:
                        tr_d = None  # use ones_r
                    elif d == 1:
                        tr_d = trp.tile([P, NI * BH], F32R, tag="tr")
                        nc.scalar.activation(
                            tr_d[:], xt[:], mybir.ActivationFunctionType.Copy
                        )
                    else:
                        t_new = statep.tile([P, NI * BH], F32, tag="state")
                        HS = NI * BH // 2
                        for hh in range(2):
                            sl = slice(hh * HS, (hh + 1) * HS)
                            prod = prodp.tile([P, HS], F32, tag="prod", name=f"prod_{h}_{d}_{hh}")
                            nc.vector.scalar_tensor_tensor(
                                prod[:], t_m1[:, sl], 2.0, xt[:, sl],
                                AluOpType.mult, AluOpType.mult,
                            )
                            if d == 2:
                                # T2 = 2*xt^2 - 1
                                nc.vector.tensor_scalar_sub(t_new[:, sl], prod[:], 1.0)
                            else:
                                nc.vector.tensor_sub(t_new[:, sl], prod[:], t_m2[:, sl])
                        t_m2, t_m1 = t_m1, t_new
                        tr_d = trp.tile([P, NI * BH], F32R, tag="tr")
                        nc.scalar.activation(
                            tr_d[:], t_new[:], mybir.ActivationFunctionType.Copy
                        )
                    if d == 1:
                        t_m2, t_m1 = xt, xt  # T1 = xt, T0 handled via scalar sub at d=2

                    # ---- W stream + matmuls for this degree ----
                    for ws in range(4):  # i-tile pairs
                        wst = wstagep.tile([P, 2 * OC], F32, tag="wstage", name=f"wst_{h}_{d}_{ws}")
                        for il in range(2):
                            it = ws * 2 + il
                            nc.sync.dma_start(
                                wst[:, il * OC : (il + 1) * OC],
                                w_d[d, it * P : (it + 1) * P, :],
                            )
                        wr = wrp.tile([P, 2 * OC], F32R, tag="wr", name=f"wr_{h}_{d}_{ws}")
                        nc.gpsimd.tensor_copy(wr[:], wst[:])
                        for il in range(2):
                            it = ws * 2 + il
                            if d == 0:
                                rhs = ones_r[:]
                            else:
                                rhs = tr_d[:, it * BH : (it + 1) * BH]
                            for ot in range(NO):
                                nc.tensor.matmul(
                                    accs[ot][:],
                                    wr[:, il * OC + ot * P : il * OC + (ot + 1) * P],
                                    rhs,
                                    start=(d == 0 and it == 0),
                                    stop=(d == DEG and it == NI - 1),
                                )

                # ---- evacuate psum -> SBUF -> y.T ----
                for ot in range(NO):
                    ev = evacp.tile([P, BH], F32, tag="evac", name=f"ev_h{h}_o{ot}")
                    nc.vector.tensor_copy(ev[:], accs[ot][:])
                    nc.sync.dma_start(
                        yt_d[ot * P : (ot + 1) * P, h * BH : (h + 1) * BH],
                        ev[:],
                    )

    nc.compile()
    return nc


_CACHE: dict = {}


def kernel(x, cheby_coeffs, tanh_scale, tanh_bias):
    x = np.ascontiguousarray(np.asarray(x, dtype=np.float32))
    w = np.ascontiguousarray(np.asarray(cheby_coeffs, dtype=np.float32))
    ts = float(np.asarray(tanh_scale))
    tb = float(np.asarray(tanh_bias))

    key = (ts, tb)
    if key not in _CACHE:
        _CACHE[key] = _build(ts, tb)
    nc = _CACHE[key]

    in_maps = [
        {"x": x[c * BC : (c + 1) * BC], "w": w} for c in range(N_CORES)
    ]
    res = bass_utils.run_bass_kernel_spmd(
        nc, in_maps, core_ids=list(range(N_CORES)), trace=False
    )

    y = np.empty((B, OC), dtype=np.float32)
    for c in range(N_CORES):
        y[c * BC : (c + 1) * BC, :] = res.results[c]["yt"].T
    return y


# revision 7
# speedup vs baseline: 2.4236x; 2.4236x over previous
"""ChebyKAN layer on 8 Trainium2 NeuronCores.

y = einsum('dbi,dio->bo', cheby_basis(tanh(x)), cheby_coeffs)

Strategy (per core, data-parallel over batch):
  - each core takes 1024 rows of x (8192/8) and the full coeffs
  - x rows are PE-transposed to [i, b] layout, tanh applied on evacuation
    (ACT writes fp32r directly)
  - Chebyshev basis built on-the-fly in fp32r on the vector engine
  - contraction as fp32r matmuls (full-rate on TRN2, ~1e-4 rel err):
    stationary = W[d, i-tile, o-tile], moving = T_d[i-tile, b-half],
    psum holds y.T chunks [o-tile 128, b-half 512] x 8 o-tiles = 8 banks
  - two b-halves of 512; W streamed from HBM once per half, cast to
    fp32r on the vector engine
  - output is y.T per core; host transposes and concatenates
"""

import numpy as np

import concourse.bass as bass
import concourse.tile as tile
from concourse import bacc, mybir
from concourse import bass_utils
from concourse import masks
from concourse.alu_op_type import AluOpType

N_CORES = 8
B = 8192
IC = 1024
OC = 1024
DEG = 8  # polynomial degree; DEG+1 = 9 basis terms
BC = B // N_CORES  # 1024 batch rows per core
P = 128
NI = IC // P  # 8 i-tiles
NO = OC // P  # 8 o-tiles
BH = BC // 2  # 512, b-half
F32 = mybir.dt.float32
F32R = mybir.dt.float32r


def _build(tanh_scale: float, tanh_bias: float):
    nc = bacc.Bacc("TRN2", target_bir_lowering=False, debug=False, num_devices=N_CORES)

    x_d = nc.dram_tensor("x", [BC, IC], F32, kind="ExternalInput").ap()
    w_d = nc.dram_tensor("w", [DEG + 1, IC, OC], F32, kind="ExternalInput").ap()
    yt_d = nc.dram_tensor("yt", [OC, BC], F32, kind="ExternalOutput").ap()

    with tile.TileContext(nc) as tc:
        with (
            tc.tile_pool(name="const", bufs=1) as constp,
            tc.tile_pool(name="xin", bufs=2) as xinp,
            tc.tile_pool(name="xt", bufs=2) as xtp,
            tc.tile_pool(name="state", bufs=3) as statep,
            tc.tile_pool(name="prod", bufs=1) as prodp,
            tc.tile_pool(name="wstage", bufs=2) as wstagep,
            tc.tile_pool(name="wr", bufs=2) as wrp,
            tc.tile_pool(name="evac", bufs=2) as evacp,
            tc.tile_pool(name="ps", bufs=8, space=bass.MemorySpace.PSUM) as psp,
        ):
            ident = constp.tile([P, P], F32)
            masks.make_identity(nc, ident[:])
            ones_f = constp.tile([P, BH], F32)
            nc.gpsimd.memset(ones_f[:], 1.0)
            ones_r = constp.tile([P, BH], F32R)
            nc.vector.tensor_copy(ones_r[:], ones_f[:])

            for h in range(2):
                # ---- x transpose + tanh -> fp32r xt in [i, b] layout ----
                # xt free layout: i_tile-major, 512 b-local each
                xt = xtp.tile([P, NI * BH], F32R, tag="xt", name=f"xt_{h}")
                for bt in range(BH // P):  # 4 b-tiles of 128 in this half
                    xnat = xinp.tile([P, IC], F32, tag="xin", name=f"xn_{h}_{bt}")
                    nc.sync.dma_start(
                        xnat[:], x_d[h * BH + bt * P : h * BH + (bt + 1) * P, :]
                    )
                    for it in range(NI):
                        ptile = psp.tile([P, 512], F32, tag="ps", name=f"pt_{h}_{bt}_{it}")
                        nc.tensor.transpose(
                            ptile[:, 0:P], xnat[:, it * P : (it + 1) * P], ident[:]
                        )
                        nc.scalar.activation(
                            xt[:, it * BH + bt * P : it * BH + (bt + 1) * P],
                            ptile[:, 0:P],
                            mybir.ActivationFunctionType.Tanh,
                            bias=tanh_bias,
                            scale=tanh_scale,
                        )

                # ---- accumulation psum tiles: y.T chunk per o-tile ----
                accs = [
                    psp.tile([P, BH], F32, tag="ps", name=f"acc_h{h}_o{ot}")
                    for ot in range(NO)
                ]

                # ---- degree loop ----
                t_m1 = xt  # T_{d-1} (fp32r slab)
                t_m2 = None  # T_{d-2}
                for d in range(DEG + 1):
                    # fp32r moving operand for this degree
                    if d == 0:
                        tr_d = None  # use ones_r
                    elif d == 1:
                        tr_d = xt
                    else:
                        t_new = statep.tile(
                            [P, NI * BH], F32R, tag="state", name=f"st_{h}_{d}"
                        )
                        HS = NI * BH // 2
                        for hh in range(2):
                            sl = slice(hh * HS, (hh + 1) * HS)
                            prod = prodp.tile(
                                [P, HS], F32R, tag="prod", name=f"prod_{h}_{d}_{hh}"
                            )
                            nc.vector.scalar_tensor_tensor(
                                prod[:],
                                t_m1[:, sl],
                                2.0,
                                xt[:, sl],
                                AluOpType.mult,
                                AluOpType.mult,
                            )
                            if d == 2:
                                # T2 = 2*xt^2 - 1
                                nc.vector.tensor_scalar_sub(t_new[:, sl], prod[:], 1.0)
                            else:
                                nc.vector.tensor_sub(t_new[:, sl], prod[:], t_m2[:, sl])
                        t_m2, t_m1 = t_m1, t_new
                        tr_d = t_new
                    if d == 1:
                        t_m2, t_m1 = xt, xt  # T1 = xt; T0 handled via scalar sub at d=2

                    # ---- W stream + matmuls for this degree ----
                    for ws in range(4):  # i-tile pairs
                        wst = wstagep.tile(
                            [P, 2 * OC], F32, tag="wstage", name=f"wst_{h}_{d}_{ws}"
                        )
                        for il in range(2):
                            it = ws * 2 + il
                            nc.sync.dma_start(
                                wst[:, il * OC : (il + 1) * OC],
                                w_d[d, it * P : (it + 1) * P, :],
                            )
                        wr = wrp.tile([P, 2 * OC], F32R, tag="wr", name=f"wr_{h}_{d}_{ws}")
                        nc.vector.tensor_copy(wr[:], wst[:])
                        for il in range(2):
                            it = ws * 2 + il
                            if d == 0:
                                rhs = ones_r[:]
                            else:
                                rhs = tr_d[:, it * BH : (it + 1) * BH]
                            for ot in range(NO):
                                nc.tensor.matmul(
                                    accs[ot][:],
                                    wr[:, il * OC + ot * P : il * OC + (ot + 1) * P],
                                    rhs,
                                    start=(d == 0 and it == 0),
                                    stop=(d == DEG and it == NI - 1),
                                )

                # ---- evacuate psum -> SBUF -> y.T ----
                for ot in range(NO):
                    ev = evacp.tile([P, BH], F32, tag="evac", name=f"ev_h{h}_o{ot}")
                    nc.scalar.activation(
                        ev[:], accs[ot][:], mybir.ActivationFunctionType.Copy
                    )
                    nc.sync.dma_start(
                        yt_d[ot * P : (ot + 1) * P, h * BH : (h + 1) * BH],
                        ev[:],
                    )

    nc.compile()
    return nc


_CACHE: dict = {}


def kernel(x, cheby_coeffs, tanh_scale, tanh_bias):
    x = np.ascontiguousarray(np.asarray(x, dtype=np.float32))
    w = np.ascontiguousarray(np.asarray(cheby_coeffs, dtype=np.float32))
    ts = float(np.asarray(tanh_scale))
    tb = float(np.asarray(tanh_bias))

    key = (ts, tb)
    if key not in _CACHE:
        _CACHE[key] = _build(ts, tb)
    nc = _CACHE[key]

    in_maps = [
        {"x": x[c * BC : (c + 1) * BC], "w": w} for c in range(N_CORES)
    ]
    res = bass_utils.run_bass_kernel_spmd(
        nc, in_maps, core_ids=list(range(N_CORES)), trace=False
    )

    y = np.empty((B, OC), dtype=np.float32)
    for c in range(N_CORES):
        y[c * BC : (c + 1) * BC, :] = res.results[c]["yt"].T
    return y


# revision 8
# speedup vs baseline: 2.4262x; 1.0011x over previous
"""ChebyKAN layer on 8 Trainium2 NeuronCores.

y = einsum('dbi,dio->bo', cheby_basis(tanh(x)), cheby_coeffs)

Strategy (per core, data-parallel over batch):
  - each core takes 1024 rows of x (8192/8) and the full coeffs
  - x rows are PE-transposed to [i, b] layout, tanh applied on evacuation
  - Chebyshev basis built on-the-fly in fp32 on the vector engine,
    rounded to fp32r on the scalar engine
  - contraction as fp32r matmuls (full-rate on TRN2, ~1e-4 rel err):
    stationary = W[d, i-tile, o-tile], moving = T_d[i-tile, b-half],
    psum holds y.T chunks [o-tile 128, b-half 512] x 8 o-tiles = 8 banks
  - two b-halves of 512; W streamed from HBM once per half, cast to
    fp32r on the vector engine
  - output is y.T per core; host transposes and concatenates
"""

import numpy as np

import concourse.bass as bass
import concourse.tile as tile
from concourse import bacc, mybir
from concourse import bass_utils
from concourse import masks
from concourse.alu_op_type import AluOpType

N_CORES = 8
B = 8192
IC = 1024
OC = 1024
DEG = 8  # polynomial degree; DEG+1 = 9 basis terms
BC = B // N_CORES  # 1024 batch rows per core
P = 128
NI = IC // P  # 8 i-tiles
NO = OC // P  # 8 o-tiles
BH = BC // 2  # 512, b-half
F32 = mybir.dt.float32
F32R = mybir.dt.float32r


def _build(tanh_scale: float, tanh_bias: float):
    nc = bacc.Bacc("TRN2", target_bir_lowering=False, debug=False, num_devices=N_CORES)

    x_d = nc.dram_tensor("x", [BC, IC], F32, kind="ExternalInput").ap()
    w_d = nc.dram_tensor("w", [DEG + 1, IC, OC], F32, kind="ExternalInput").ap()
    yt_d = nc.dram_tensor("yt", [OC, BC], F32, kind="ExternalOutput").ap()

    with tile.TileContext(nc) as tc:
        with (
            tc.tile_pool(name="const", bufs=1) as constp,
            tc.tile_pool(name="xin", bufs=2) as xinp,
            tc.tile_pool(name="xt", bufs=2) as xtp,
            tc.tile_pool(name="state", bufs=3) as statep,
            tc.tile_pool(name="prod", bufs=1) as prodp,
            tc.tile_pool(name="tr", bufs=2) as trp,
            tc.tile_pool(name="wstage", bufs=2) as wstagep,
            tc.tile_pool(name="wr", bufs=2) as wrp,
            tc.tile_pool(name="evac", bufs=2) as evacp,
            tc.tile_pool(name="ps", bufs=8, space=bass.MemorySpace.PSUM) as psp,
        ):
            ident = constp.tile([P, P], F32)
            masks.make_identity(nc, ident[:])
            ones_f = constp.tile([P, BH], F32)
            nc.gpsimd.memset(ones_f[:], 1.0)
            ones_r = constp.tile([P, BH], F32R)
            nc.vector.tensor_copy(ones_r[:], ones_f[:])

            for h in range(2):
                # ---- x transpose + tanh -> fp32r xt in [i, b] layout ----
                # xt free layout: i_tile-major, 512 b-local each
                xt = xtp.tile([P, NI * BH], F32, tag="xt", name=f"xt_{h}")
                for bt in range(BH // P):  # 4 b-tiles of 128 in this half
                    xnat = xinp.tile([P, IC], F32, tag="xin", name=f"xn_{h}_{bt}")
                    nc.sync.dma_start(
                        xnat[:], x_d[h * BH + bt * P : h * BH + (bt + 1) * P, :]
                    )
                    for it in range(NI):
                        ptile = psp.tile([P, 512], F32, tag="ps", name=f"pt_{h}_{bt}_{it}")
                        nc.tensor.transpose(
                            ptile[:, 0:P], xnat[:, it * P : (it + 1) * P], ident[:]
                        )
                        nc.scalar.activation(
                            xt[:, it * BH + bt * P : it * BH + (bt + 1) * P],
                            ptile[:, 0:P],
                            mybir.ActivationFunctionType.Tanh,
                            bias=tanh_bias,
                            scale=tanh_scale,
                        )

                # ---- accumulation psum tiles: y.T chunk per o-tile ----
                accs = [
                    psp.tile([P, BH], F32, tag="ps", name=f"acc_h{h}_o{ot}")
                    for ot in range(NO)
                ]

                # ---- degree loop ----
                t_m1 = xt  # T_{d-1} (fp32r slab)
                t_m2 = None  # T_{d-2}
                for d in range(DEG + 1):
                    # fp32r moving operand for this degree
                    if d == 0:
                        tr_d = None  # use ones_r
                    elif d == 1:
                        tr_d = trp.tile([P, NI * BH], F32R, tag="tr", name=f"tr_{h}_1")
                        nc.scalar.activation(
                            tr_d[:], xt[:], mybir.ActivationFunctionType.Copy
                        )
                    else:
                        t_new = statep.tile(
                            [P, NI * BH], F32, tag="state", name=f"st_{h}_{d}"
                        )
                        HS = NI * BH // 2
                        for hh in range(2):
                            sl = slice(hh * HS, (hh + 1) * HS)
                            prod = prodp.tile(
                                [P, HS], F32, tag="prod", name=f"prod_{h}_{d}_{hh}"
                            )
                            nc.vector.scalar_tensor_tensor(
                                prod[:],
                                t_m1[:, sl],
                                2.0,
                                xt[:, sl],
                                AluOpType.mult,
                                AluOpType.mult,
                            )
                            if d == 2:
                                # T2 = 2*xt^2 - 1
                                nc.vector.tensor_scalar_sub(t_new[:, sl], prod[:], 1.0)
                            else:
                                nc.vector.tensor_sub(t_new[:, sl], prod[:], t_m2[:, sl])
                        t_m2, t_m1 = t_m1, t_new
                        tr_d = trp.tile([P, NI * BH], F32R, tag="tr", name=f"tr_{h}_{d}")
                        nc.scalar.activation(
                            tr_d[:], t_new[:], mybir.ActivationFunctionType.Copy
                        )
                    if d == 1:
                        t_m2, t_m1 = xt, xt  # T1 = xt; T0 handled via scalar sub at d=2

                    # ---- W stream + matmuls for this degree ----
                    for ws in range(4):  # i-tile pairs
                        wst = wstagep.tile(
                            [P, 2 * OC], F32, tag="wstage", name=f"wst_{h}_{d}_{ws}"
                        )
                        for il in range(2):
                            it = ws * 2 + il
                            nc.sync.dma_start(
                                wst[:, il * OC : (il + 1) * OC],
                                w_d[d, it * P : (it + 1) * P, :],
                            )
                        wr = wrp.tile([P, 2 * OC], F32R, tag="wr", name=f"wr_{h}_{d}_{ws}")
                        nc.vector.tensor_copy(wr[:], wst[:])
                        for il in range(2):
                            it = ws * 2 + il
                            if d == 0:
                                rhs = ones_r[:]
                            else:
                                rhs = tr_d[:, it * BH : (it + 1) * BH]
                            for ot in range(NO):
                                nc.tensor.matmul(
                                    accs[ot][:],
                                    wr[:, il * OC + ot * P : il * OC + (ot + 1) * P],
                                    rhs,
                                    start=(d == 0 and it == 0),
                                    stop=(d == DEG and it == NI - 1),
                                )

                # ---- evacuate psum -> SBUF -> y.T ----
                for ot in range(NO):
                    ev = evacp.tile([P, BH], F32, tag="evac", name=f"ev_h{h}_o{ot}")
                    nc.scalar.activation(
                        ev[:], accs[ot][:], mybir.ActivationFunctionType.Copy
                    )
                    nc.sync.dma_start(
                        yt_d[ot * P : (ot + 1) * P, h * BH : (h + 1) * BH],
                        ev[:],
                    )

    nc.compile()
    return nc


_CACHE: dict = {}


def kernel(x, cheby_coeffs, tanh_scale, tanh_bias):
    x = np.ascontiguousarray(np.asarray(x, dtype=np.float32))
    w = np.ascontiguousarray(np.asarray(cheby_coeffs, dtype=np.float32))
    ts = float(np.asarray(tanh_scale))
    tb = float(np.asarray(tanh_bias))

    key = (ts, tb)
    if key not in _CACHE:
        _CACHE[key] = _build(ts, tb)
    nc = _CACHE[key]

    in_maps = [
        {"x": x[c * BC : (c + 1) * BC], "w": w} for c in range(N_CORES)
    ]
    res = bass_utils.run_bass_kernel_spmd(
        nc, in_maps, core_ids=list(range(N_CORES)), trace=False
    )

    y = np.empty((B, OC), dtype=np.float32)
    for c in range(N_CORES):
        y[c * BC : (c + 1) * BC, :] = res.results[c]["yt"].T
    return y


# revision 10
# speedup vs baseline: 2.4269x; 1.0003x over previous
"""ChebyKAN layer on 8 Trainium2 NeuronCores.

y = einsum('dbi,dio->bo', cheby_basis(tanh(x)), cheby_coeffs)

Strategy (per core, data-parallel over batch):
  - each core takes 1024 rows of x (8192/8) and the full coeffs
  - x arrives pre-transposed ([i, b] layout) from the host; tanh on the
    scalar engine
  - Chebyshev basis built on-the-fly in fp32 on the vector engine,
    rounded to fp32r on the scalar engine
  - contraction as fp32r matmuls (full-rate on TRN2, ~1e-4 rel err):
    stationary = W[d, i-tile, o-tile], moving = T_d[i-tile, b-half],
    psum holds y.T chunks [o-tile 128, b-half 512] x 8 o-tiles = 8 banks
  - two b-halves of 512; W streamed from HBM once per half, cast to
    fp32r on the vector engine
  - output is y.T per core; host transposes and concatenates
"""

import numpy as np

import concourse.bass as bass
import concourse.tile as tile
from concourse import bacc, mybir
from concourse import bass_utils
from concourse import masks
from concourse.alu_op_type import AluOpType

N_CORES = 8
B = 8192
IC = 1024
OC = 1024
DEG = 8  # polynomial degree; DEG+1 = 9 basis terms
BC = B // N_CORES  # 1024 batch rows per core
P = 128
NI = IC // P  # 8 i-tiles
NO = OC // P  # 8 o-tiles
BH = BC // 2  # 512, b-half
F32 = mybir.dt.float32
F32R = mybir.dt.float32r


def _build(tanh_scale: float, tanh_bias: float):
    nc = bacc.Bacc("TRN2", target_bir_lowering=False, debug=False, num_devices=N_CORES)

    xT_d = nc.dram_tensor("xT", [IC, BC], F32, kind="ExternalInput").ap()
    w_d = nc.dram_tensor("w", [DEG + 1, IC, OC], F32, kind="ExternalInput").ap()
    yt_d = nc.dram_tensor("yt", [OC, BC], F32, kind="ExternalOutput").ap()

    with tile.TileContext(nc) as tc:
        with (
            tc.tile_pool(name="const", bufs=1) as constp,
            tc.tile_pool(name="xin", bufs=2) as xinp,
            tc.tile_pool(name="xt", bufs=2) as xtp,
            tc.tile_pool(name="state", bufs=3) as statep,
            tc.tile_pool(name="prod", bufs=1) as prodp,
            tc.tile_pool(name="tr", bufs=2) as trp,
            tc.tile_pool(name="wstage", bufs=2) as wstagep,
            tc.tile_pool(name="wr", bufs=2) as wrp,
            tc.tile_pool(name="evac", bufs=2) as evacp,
            tc.tile_pool(name="ps", bufs=8, space=bass.MemorySpace.PSUM) as psp,
        ):
            ones_f = constp.tile([P, BH], F32)
            nc.gpsimd.memset(ones_f[:], 1.0)
            ones_r = constp.tile([P, BH], F32R)
            nc.vector.tensor_copy(ones_r[:], ones_f[:])

            for h in range(2):
                # ---- load x.T slices + tanh -> fp32 xt ----
                # xt free layout: i_tile-major, 512 b-local each
                xt = xtp.tile([P, NI * BH], F32, tag="xt", name=f"xt_{h}")
                for it in range(NI):
                    xst = xinp.tile([P, BH], F32, tag="xin", name=f"xs_{h}_{it}")
                    nc.sync.dma_start(
                        xst[:], xT_d[it * P : (it + 1) * P, h * BH : (h + 1) * BH]
                    )
                    nc.scalar.activation(
                        xt[:, it * BH : (it + 1) * BH],
                        xst[:],
                        mybir.ActivationFunctionType.Tanh,
                        bias=tanh_bias,
                        scale=tanh_scale,
                    )

                # ---- accumulation psum tiles: y.T chunk per o-tile ----
                accs = [
                    psp.tile([P, BH], F32, tag="ps", name=f"acc_h{h}_o{ot}")
                    for ot in range(NO)
                ]

                # ---- degree loop ----
                t_m1 = xt  # T_{d-1} (fp32r slab)
                t_m2 = None  # T_{d-2}
                for d in range(DEG + 1):
                    # fp32r moving operand for this degree
                    if d == 0:
                        tr_d = None  # use ones_r
                    elif d == 1:
                        tr_d = trp.tile([P, NI * BH], F32R, tag="tr", name=f"tr_{h}_1")
                        nc.scalar.activation(
                            tr_d[:], xt[:], mybir.ActivationFunctionType.Copy
                        )
                    else:
                        t_new = statep.tile(
                            [P, NI * BH], F32, tag="state", name=f"st_{h}_{d}"
                        )
                        HS = NI * BH // 2
                        for hh in range(2):
                            sl = slice(hh * HS, (hh + 1) * HS)
                            prod = prodp.tile(
                                [P, HS], F32, tag="prod", name=f"prod_{h}_{d}_{hh}"
                            )
                            nc.vector.scalar_tensor_tensor(
                                prod[:],
                                t_m1[:, sl],
                                2.0,
                                xt[:, sl],
                                AluOpType.mult,
                                AluOpType.mult,
                            )
                            if d == 2:
                                # T2 = 2*xt^2 - 1
                                nc.vector.tensor_scalar_sub(t_new[:, sl], prod[:], 1.0)
                            else:
                                nc.vector.tensor_sub(t_new[:, sl], prod[:], t_m2[:, sl])
                        t_m2, t_m1 = t_m1, t_new
                        tr_d = trp.tile([P, NI * BH], F32R, tag="tr", name=f"tr_{h}_{d}")
                        nc.scalar.activation(
                            tr_d[:], t_new[:], mybir.ActivationFunctionType.Copy
                        )
                    if d == 1:
                        t_m2, t_m1 = xt, xt  # T1 = xt; T0 handled via scalar sub at d=2

                    # ---- W stream + matmuls for this degree ----
                    for ws in range(4):  # i-tile pairs
                        wst = wstagep.tile(
                            [P, 2 * OC], F32, tag="wstage", name=f"wst_{h}_{d}_{ws}"
                        )
                        for il in range(2):
                            it = ws * 2 + il
                            nc.sync.dma_start(
                                wst[:, il * OC : (il + 1) * OC],
                                w_d[d, it * P : (it + 1) * P, :],
                            )
                        wr = wrp.tile([P, 2 * OC], F32R, tag="wr", name=f"wr_{h}_{d}_{ws}")
                        nc.vector.tensor_copy(wr[:], wst[:])
                        for il in range(2):
                            it = ws * 2 + il
                            if d == 0:
                                rhs = ones_r[:]
                            else:
                                rhs = tr_d[:, it * BH : (it + 1) * BH]
                            for ot in range(NO):
                                nc.tensor.matmul(
                                    accs[ot][:],
                                    wr[:, il * OC + ot * P : il * OC + (ot + 1) * P],
                                    rhs,
                                    start=(d == 0 and it == 0),
                                    stop=(d == DEG and it == NI - 1),
                                )

                # ---- evacuate psum -> SBUF -> y.T ----
                for ot in range(NO):
                    ev = evacp.tile([P, BH], F32, tag="evac", name=f"ev_h{h}_o{ot}")
                    nc.scalar.activation(
                        ev[:], accs[ot][:], mybir.ActivationFunctionType.Copy
                    )
                    nc.sync.dma_start(
                        yt_d[ot * P : (ot + 1) * P, h * BH : (h + 1) * BH],
                        ev[:],
                    )

    nc.compile()
    return nc


_CACHE: dict = {}


def make_in_maps(x, w):
    return [
        {"xT": np.ascontiguousarray(x[c * BC : (c + 1) * BC].T), "w": w}
        for c in range(N_CORES)
    ]


def kernel(x, cheby_coeffs, tanh_scale, tanh_bias):
    x = np.ascontiguousarray(np.asarray(x, dtype=np.float32))
    w = np.ascontiguousarray(np.asarray(cheby_coeffs, dtype=np.float32))
    ts = float(np.asarray(tanh_scale))
    tb = float(np.asarray(tanh_bias))

    key = (ts, tb)
    if key not in _CACHE:
        _CACHE[key] = _build(ts, tb)
    nc = _CACHE[key]

    in_maps = make_in_maps(x, w)
    res = bass_utils.run_bass_kernel_spmd(
        nc, in_maps, core_ids=list(range(N_CORES)), trace=False
    )

    y = np.empty((B, OC), dtype=np.float32)
    for c in range(N_CORES):
        y[c * BC : (c + 1) * BC, :] = res.results[c]["yt"].T
    return y


# revision 11
# speedup vs baseline: 2.4336x; 1.0028x over previous
"""ChebyKAN layer on 8 Trainium2 NeuronCores.

y = einsum('dbi,dio->bo', cheby_basis(tanh(x)), cheby_coeffs)

Strategy (per core, data-parallel over batch):
  - each core takes 1024 rows of x (8192/8) and the full coeffs
  - x arrives pre-transposed ([i, b] layout) from the host; tanh on the
    scalar engine
  - Chebyshev basis built on-the-fly in fp32 on the vector engine,
    rounded to fp32r on the scalar engine
  - contraction as fp32r matmuls (full-rate on TRN2, ~1e-4 rel err):
    stationary = W[d, i-tile, o-tile], moving = T_d[i-tile, b-half],
    psum holds y.T chunks [o-tile 128, b-half 512] x 8 o-tiles = 8 banks
  - two b-halves of 512; W streamed from HBM once per half, cast to
    fp32r on the vector engine
  - output is y.T per core; host transposes and concatenates
"""

import numpy as np

import concourse.bass as bass
import concourse.tile as tile
from concourse import bacc, mybir
from concourse import bass_utils
from concourse import masks
from concourse.alu_op_type import AluOpType

N_CORES = 8
B = 8192
IC = 1024
OC = 1024
DEG = 8  # polynomial degree; DEG+1 = 9 basis terms
BC = B // N_CORES  # 1024 batch rows per core
P = 128
NI = IC // P  # 8 i-tiles
NO = OC // P  # 8 o-tiles
BH = BC // 2  # 512, b-half
F32 = mybir.dt.float32
F32R = mybir.dt.float32r


def _build(tanh_scale: float, tanh_bias: float):
    nc = bacc.Bacc("TRN2", target_bir_lowering=False, debug=False, num_devices=N_CORES)

    xT_d = nc.dram_tensor("xT", [IC, BC], F32, kind="ExternalInput").ap()
    w_d = nc.dram_tensor("w", [DEG + 1, IC, OC], F32, kind="ExternalInput").ap()
    yt_d = nc.dram_tensor("yt", [OC, BC], F32, kind="ExternalOutput").ap()

    with tile.TileContext(nc) as tc:
        with (
            tc.tile_pool(name="const", bufs=1) as constp,
            tc.tile_pool(name="xin", bufs=2) as xinp,
            tc.tile_pool(name="xt", bufs=2) as xtp,
            tc.tile_pool(name="state", bufs=3) as statep,
            tc.tile_pool(name="prod", bufs=1) as prodp,
            tc.tile_pool(name="tr", bufs=2) as trp,
            tc.tile_pool(name="wstage", bufs=3) as wstagep,
            tc.tile_pool(name="wr", bufs=3) as wrp,
            tc.tile_pool(name="evac", bufs=2) as evacp,
            tc.tile_pool(name="ps", bufs=8, space=bass.MemorySpace.PSUM) as psp,
        ):
            ones_f = constp.tile([P, BH], F32)
            nc.gpsimd.memset(ones_f[:], 1.0)
            ones_r = constp.tile([P, BH], F32R)
            nc.vector.tensor_copy(ones_r[:], ones_f[:])

            for h in range(2):
                # ---- load x.T slices + tanh -> fp32 xt ----
                # xt free layout: i_tile-major, 512 b-local each
                xt = xtp.tile([P, NI * BH], F32, tag="xt", name=f"xt_{h}")
                for it in range(NI):
                    xst = xinp.tile([P, BH], F32, tag="xin", name=f"xs_{h}_{it}")
                    nc.sync.dma_start(
                        xst[:], xT_d[it * P : (it + 1) * P, h * BH : (h + 1) * BH]
                    )
                    nc.scalar.activation(
                        xt[:, it * BH : (it + 1) * BH],
                        xst[:],
                        mybir.ActivationFunctionType.Tanh,
                        bias=tanh_bias,
                        scale=tanh_scale,
                    )

                # ---- accumulation psum tiles: y.T chunk per o-tile ----
                accs = [
                    psp.tile([P, BH], F32, tag="ps", name=f"acc_h{h}_o{ot}")
                    for ot in range(NO)
                ]

                # ---- degree loop ----
                t_m1 = xt  # T_{d-1} (fp32r slab)
                t_m2 = None  # T_{d-2}
                for d in range(DEG + 1):
                    # fp32r moving operand for this degree
                    if d == 0:
                        tr_d = None  # use ones_r
                    elif d == 1:
                        tr_d = trp.tile([P, NI * BH], F32R, tag="tr", name=f"tr_{h}_1")
                        nc.scalar.activation(
                            tr_d[:], xt[:], mybir.ActivationFunctionType.Copy
                        )
                    else:
                        t_new = statep.tile(
                            [P, NI * BH], F32, tag="state", name=f"st_{h}_{d}"
                        )
                        HS = NI * BH // 2
                        for hh in range(2):
                            sl = slice(hh * HS, (hh + 1) * HS)
                            prod = prodp.tile(
                                [P, HS], F32, tag="prod", name=f"prod_{h}_{d}_{hh}"
                            )
                            nc.vector.scalar_tensor_tensor(
                                prod[:],
                                t_m1[:, sl],
                                2.0,
                                xt[:, sl],
                                AluOpType.mult,
                                AluOpType.mult,
                            )
                            if d == 2:
                                # T2 = 2*xt^2 - 1
                                nc.vector.tensor_scalar_sub(t_new[:, sl], prod[:], 1.0)
                            else:
                                nc.vector.tensor_sub(t_new[:, sl], prod[:], t_m2[:, sl])
                        t_m2, t_m1 = t_m1, t_new
                        tr_d = trp.tile([P, NI * BH], F32R, tag="tr", name=f"tr_{h}_{d}")
                        nc.scalar.activation(
                            tr_d[:], t_new[:], mybir.ActivationFunctionType.Copy
                        )
                    if d == 1:
                        t_m2, t_m1 = xt, xt  # T1 = xt; T0 handled via scalar sub at d=2

                    # ---- W stream + matmuls for this degree ----
                    # d==0 uses single-i-tile slabs: shorter critical chain
                    # from W DMA + cast to the first matmul of the half
                    ntiles_per_ws = 1 if d == 0 else 2
                    for ws in range(NI // ntiles_per_ws):
                        wst = wstagep.tile(
                            [P, ntiles_per_ws * OC],
                            F32,
                            tag="wstage",
                            name=f"wst_{h}_{d}_{ws}",
                        )
                        for il in range(ntiles_per_ws):
                            it = ws * ntiles_per_ws + il
                            nc.sync.dma_start(
                                wst[:, il * OC : (il + 1) * OC],
                                w_d[d, it * P : (it + 1) * P, :],
                            )
                        wr = wrp.tile(
                            [P, ntiles_per_ws * OC],
                            F32R,
                            tag="wr",
                            name=f"wr_{h}_{d}_{ws}",
                        )
                        nc.vector.tensor_copy(wr[:], wst[:])
                        for il in range(ntiles_per_ws):
                            it = ws * ntiles_per_ws + il
                            if d == 0:
                                rhs = ones_r[:]
                            else:
                                rhs = tr_d[:, it * BH : (it + 1) * BH]
                            for ot in range(NO):
                                nc.tensor.matmul(
                                    accs[ot][:],
                                    wr[:, il * OC + ot * P : il * OC + (ot + 1) * P],
                                    rhs,
                                    start=(d == 0 and it == 0),
                                    stop=(d == DEG and it == NI - 1),
                                )

                # ---- evacuate psum -> SBUF -> y.T ----
                for ot in range(NO):
                    ev = evacp.tile([P, BH], F32, tag="evac", name=f"ev_h{h}_o{ot}")
                    nc.scalar.activation(
                        ev[:], accs[ot][:], mybir.ActivationFunctionType.Copy
                    )
                    nc.sync.dma_start(
                        yt_d[ot * P : (ot + 1) * P, h * BH : (h + 1) * BH],
                        ev[:],
                    )

    nc.compile()
    return nc


_CACHE: dict = {}


def make_in_maps(x, w):
    return [
        {"xT": np.ascontiguousarray(x[c * BC : (c + 1) * BC].T), "w": w}
        for c in range(N_CORES)
    ]


def kernel(x, cheby_coeffs, tanh_scale, tanh_bias):
    x = np.ascontiguousarray(np.asarray(x, dtype=np.float32))
    w = np.ascontiguousarray(np.asarray(cheby_coeffs, dtype=np.float32))
    ts = float(np.asarray(tanh_scale))
    tb = float(np.asarray(tanh_bias))

    key = (ts, tb)
    if key not in _CACHE:
        _CACHE[key] = _build(ts, tb)
    nc = _CACHE[key]

    in_maps = make_in_maps(x, w)
    res = bass_utils.run_bass_kernel_spmd(
        nc, in_maps, core_ids=list(range(N_CORES)), trace=False
    )

    y = np.empty((B, OC), dtype=np.float32)
    for c in range(N_CORES):
        y[c * BC : (c + 1) * BC, :] = res.results[c]["yt"].T
    return y


# revision 12
# speedup vs baseline: 2.4839x; 1.0207x over previous
"""ChebyKAN layer on 8 Trainium2 NeuronCores.

y = einsum('dbi,dio->bo', cheby_basis(tanh(x)), cheby_coeffs)

Strategy (per core, data-parallel over batch):
  - each core takes 1024 rows of x (8192/8) and the full coeffs
  - x arrives pre-transposed ([i, b] layout) from the host; tanh on the
    scalar engine
  - Chebyshev basis built on-the-fly in fp32 on the vector engine,
    rounded to fp32r on the scalar engine
  - contraction as fp32r matmuls (full-rate on TRN2, ~1e-4 rel err):
    stationary = W[d, i-tile, o-tile], moving = T_d[i-tile, b-half],
    psum holds y.T chunks [o-tile 128, b-half 512] x 8 o-tiles = 8 banks
  - two b-halves of 512; W streamed from HBM once per half, cast to
    fp32r on the vector engine
  - output is y.T per core; host transposes and concatenates
"""

import numpy as np

import concourse.bass as bass
import concourse.tile as tile
from concourse import bacc, mybir
from concourse import bass_utils
from concourse.alu_op_type import AluOpType

N_CORES = 8
B = 8192
IC = 1024
OC = 1024
DEG = 8  # polynomial degree; DEG+1 = 9 basis terms
BC = B // N_CORES  # 1024 batch rows per core
P = 128
NI = IC // P  # 8 i-tiles
NO = OC // P  # 8 o-tiles
BH = BC // 2  # 512, b-half
F32 = mybir.dt.float32
F32R = mybir.dt.float32r

# W slab granularity (in i-tiles) per degree: the first degree of a half
# uses small slabs so the first matmul's W-DMA + cast chain is short.
_D0_SLABS = [1, 1, 2, 2, 2]
_D_SLABS = [2, 2, 2, 2]


def _build(tanh_scale: float, tanh_bias: float):
    nc = bacc.Bacc("TRN2", target_bir_lowering=False, debug=False, num_devices=N_CORES)

    xT_d = nc.dram_tensor("xT", [IC, BC], F32, kind="ExternalInput").ap()
    w_d = nc.dram_tensor("w", [DEG + 1, IC, OC], F32, kind="ExternalInput").ap()
    yt_d = nc.dram_tensor("yt", [OC, BC], F32, kind="ExternalOutput").ap()

    with tile.TileContext(nc) as tc:
        with (
            tc.tile_pool(name="const", bufs=1) as constp,
            tc.tile_pool(name="xin", bufs=4) as xinp,
            tc.tile_pool(name="xt", bufs=2) as xtp,
            tc.tile_pool(name="state", bufs=3) as statep,
            tc.tile_pool(name="prod", bufs=1) as prodp,
            tc.tile_pool(name="tr", bufs=2) as trp,
            tc.tile_pool(name="wstage", bufs=3) as wstagep,
            tc.tile_pool(name="wr", bufs=3) as wrp,
            tc.tile_pool(name="evac", bufs=2) as evacp,
            tc.tile_pool(name="ps", bufs=8, space=bass.MemorySpace.PSUM) as psp,
        ):
            ones_f = constp.tile([P, BH], F32)
            nc.vector.memset(ones_f[:], 1.0)
            ones_r = constp.tile([P, BH], F32R)
            nc.vector.tensor_copy(ones_r[:], ones_f[:])

            def emit_w_slabs(h, d, slab_sizes):
                """DMA W[d] i-tile slabs and cast to fp32r; returns
                [(first_it, ntiles, wr_tile), ...]."""
                out = []
                it0 = 0
                for ws, nt in enumerate(slab_sizes):
                    wst = wstagep.tile(
                        [P, nt * OC], F32, tag="wstage", name=f"wst_{h}_{d}_{ws}"
                    )
                    for il in range(nt):
                        it = it0 + il
                        nc.sync.dma_start(
                            wst[:, il * OC : (il + 1) * OC],
                            w_d[d, it * P : (it + 1) * P, :],
                        )
                    wr = wrp.tile([P, nt * OC], F32R, tag="wr", name=f"wr_{h}_{d}_{ws}")
                    nc.vector.tensor_copy(wr[:], wst[:])
                    out.append((it0, nt, wr))
                    it0 += nt
                return out

            def emit_matmuls(accs, wr_slabs, d, tr_d):
                for it0, nt, wr in wr_slabs:
                    for il in range(nt):
                        it = it0 + il
                        if d == 0:
                            rhs = ones_r[:]
                        else:
                            rhs = tr_d[:, it * BH : (it + 1) * BH]
                        for ot in range(NO):
                            nc.tensor.matmul(
                                accs[ot][:],
                                wr[:, il * OC + ot * P : il * OC + (ot + 1) * P],
                                rhs,
                                start=(d == 0 and it == 0),
                                stop=(d == DEG and it == NI - 1),
                            )

            # W for the very first degree goes ahead of everything so the
            # PE starts as early as possible.
            d0_slabs_h0 = emit_w_slabs(0, 0, _D0_SLABS)

            # ---- load x.T slices + tanh -> fp32 xt (both halves) ----
            # xt free layout: i_tile-major, 512 b-local each
            xts = []
            for h in range(2):
                xt = xtp.tile([P, NI * BH], F32, tag="xt", name=f"xt_{h}")
                for it in range(NI):
                    xst = xinp.tile([P, BH], F32, tag="xin", name=f"xs_{h}_{it}")
                    nc.sync.dma_start(
                        xst[:], xT_d[it * P : (it + 1) * P, h * BH : (h + 1) * BH]
                    )
                    nc.scalar.activation(
                        xt[:, it * BH : (it + 1) * BH],
                        xst[:],
                        mybir.ActivationFunctionType.Tanh,
                        bias=tanh_bias,
                        scale=tanh_scale,
                    )
                xts.append(xt)

            for h in range(2):
                xt = xts[h]
                # ---- accumulation psum tiles: y.T chunk per o-tile ----
                accs = [
                    psp.tile([P, BH], F32, tag="ps", name=f"acc_h{h}_o{ot}")
                    for ot in range(NO)
                ]

                # ---- degree loop ----
                t_m1 = xt  # T_{d-1} (fp32 slab)
                t_m2 = None  # T_{d-2}
                for d in range(DEG + 1):
                    # fp32r moving operand for this degree
                    if d == 0:
                        tr_d = None  # use ones_r
                    elif d == 1:
                        tr_d = trp.tile([P, NI * BH], F32R, tag="tr", name=f"tr_{h}_1")
                        nc.scalar.activation(
                            tr_d[:], xt[:], mybir.ActivationFunctionType.Copy
                        )
                    else:
                        t_new = statep.tile(
                            [P, NI * BH], F32, tag="state", name=f"st_{h}_{d}"
                        )
                        HS = NI * BH // 2
                        for hh in range(2):
                            sl = slice(hh * HS, (hh + 1) * HS)
                            prod = prodp.tile(
                                [P, HS], F32, tag="prod", name=f"prod_{h}_{d}_{hh}"
                            )
                            nc.vector.scalar_tensor_tensor(
                                prod[:],
                                t_m1[:, sl],
                                2.0,
                                xt[:, sl],
                                AluOpType.mult,
                                AluOpType.mult,
                            )
                            if d == 2:
                                # T2 = 2*xt^2 - 1
                                nc.vector.tensor_scalar_sub(t_new[:, sl], prod[:], 1.0)
                            else:
                                nc.vector.tensor_sub(t_new[:, sl], prod[:], t_m2[:, sl])
                        t_m2, t_m1 = t_m1, t_new
                        tr_d = trp.tile([P, NI * BH], F32R, tag="tr", name=f"tr_{h}_{d}")
                        nc.scalar.activation(
                            tr_d[:], t_new[:], mybir.ActivationFunctionType.Copy
                        )
                    if d == 1:
                        t_m2, t_m1 = xt, xt  # T1 = xt; T0 handled via scalar sub at d=2

                    # ---- W stream + matmuls for this degree ----
                    if h == 0 and d == 0:
                        wr_slabs = d0_slabs_h0
                    else:
                        wr_slabs = emit_w_slabs(h, d, _D0_SLABS if d == 0 else _D_SLABS)
                    emit_matmuls(accs, wr_slabs, d, tr_d)

                # ---- evacuate psum -> SBUF -> y.T ----
                for ot in range(NO):
                    ev = evacp.tile([P, BH], F32, tag="evac", name=f"ev_h{h}_o{ot}")
                    nc.scalar.activation(
                        ev[:], accs[ot][:], mybir.ActivationFunctionType.Copy
                    )
                    nc.sync.dma_start(
                        yt_d[ot * P : (ot + 1) * P, h * BH : (h + 1) * BH],
                        ev[:],
                    )

    nc.compile()
    return nc


_CACHE: dict = {}


def make_in_maps(x, w):
    return [
        {"xT": np.ascontiguousarray(x[c * BC : (c + 1) * BC].T), "w": w}
        for c in range(N_CORES)
    ]


def kernel(x, cheby_coeffs, tanh_scale, tanh_bias):
    x = np.ascontiguousarray(np.asarray(x, dtype=np.float32))
    w = np.ascontiguousarray(np.asarray(cheby_coeffs, dtype=np.float32))
    ts = float(np.asarray(tanh_scale))
    tb = float(np.asarray(tanh_bias))

    key = (ts, tb)
    if key not in _CACHE:
        _CACHE[key] = _build(ts, tb)
    nc = _CACHE[key]

    in_maps = make_in_maps(x, w)
    res = bass_utils.run_bass_kernel_spmd(
        nc, in_maps, core_ids=list(range(N_CORES)), trace=False
    )

    y = np.empty((B, OC), dtype=np.float32)
    for c in range(N_CORES):
        y[c * BC : (c + 1) * BC, :] = res.results[c]["yt"].T
    return y
